# revision 20
# baseline (speedup 1.0000x reference)
"""AlternatingDiffHead Trainium2 kernel.

Data-parallel over batch: B=8 batch elements -> 8 NeuronCores, one batch
element per core, no collectives.

Per-core math (T=2048, C=1024, HS=128, 2 terms):
  v  = x @ Wv                                  [T, 256]
  qn = rope(x @ Wqn * 1/sqrt(HS)),  kn = rope(x @ Wkn)     [T, 128]
  Sn = qn @ kn^T  (causal)                      [T, T]
  En = exp(Sn)    (no max-sub; S is O(1))       rowsum -> ln
  D  = E0 + beta E1,  beta[t] = (c1 l0[t]) / (c0 l1[t])
  out[t] = (c0 / l0[t]) * (D @ v)[t]
where c0 = lam0, c1 = -lam1 (host-computed scalars).

v3 design notes:
 - i-major single pass: full q/k projection+RoPE first (PE stays dense),
   then one loop over the 16 row-tiles doing scores(term0+term1) -> exp
   -> beta -> D -> D^T -> AV -> out, with v-projection interleaved.
 - AV is software-pipelined one row-tile behind scores so the PE never
   waits on the exp -> beta -> D -> D^T chain.
 - D^T produced by ONE dma_start_transpose (XBAR) per row-tile into a
   [128, 16, 128] tile, replacing 136 PE transposes + PSUM->SBUF copies.
 - D combine is ONE scalar_tensor_tensor per row-tile:
   D = (E1 * beta) + E0.
 - RoPE rotate-half swap moved AFTER the sin-multiply (swap the product,
   not the input), so the PSUM->bf16 cast is fused into the cos/sin muls
   (vector reads PSUM directly); sin sign pattern pre-swapped on host.
 - DMAs split across the two hwdge queues (sync + scalar): x is loaded
   4 chunks per queue in parallel; RoPE swaps go to the scalar queue
   (idle during projection), D-transposes + output to sync.
 - Engine balance in the loop: PE scores/v/AV; ACT exp only; DVE
   D-combine, softmax stats, v copies, out scale.
"""

import numpy as np
import ml_dtypes
from contextlib import ExitStack

import concourse.bass as bass
import concourse.tile as tile
from concourse import bacc, mybir

B, T, C, HS, NT = 8, 2048, 1024, 128, 2
E2 = 2 * HS  # v/out feature dim (256)
THETA = 10000.0
NEG = -30.0
BF16, F32 = mybir.dt.bfloat16, mybir.dt.float32
AF = mybir.ActivationFunctionType
ALU = mybir.AluOpType
NCC = C // 128         # 8 contraction chunks
NTILE = T // 128       # 16 row tiles


def build_nc():
    nc = bacc.Bacc("TRN2", target_bir_lowering=False, debug=False, num_devices=8)

    xT = nc.declare_dram_parameter("xT", [C, T], BF16, isOutput=False)
    wqk = nc.declare_dram_parameter("wqk", [C, 4 * HS], BF16, isOutput=False)
    wv = nc.declare_dram_parameter("wv", [C, E2], BF16, isOutput=False)
    cosb = nc.declare_dram_parameter("cosb", [HS, T], BF16, isOutput=False)
    sinb = nc.declare_dram_parameter("sinb", [HS, T], BF16, isOutput=False)
    cmask = nc.declare_dram_parameter("cmask", [HS, 2 * HS], BF16, isOutput=False)
    lamc = nc.declare_dram_parameter("lamc", [HS, 2], F32, isOutput=False)
    outp = nc.declare_dram_parameter("out", [T, E2], F32, isOutput=True)

    with tile.TileContext(nc) as tc:
        with ExitStack() as ctx:
            pers = ctx.enter_context(tc.tile_pool(name="pers", bufs=1))
            # psA: v-proj + AV accum ([128,512] f32 = 1 bank x 2)
            psA = ctx.enter_context(
                tc.tile_pool(name="psA", bufs=2, space="PSUM")
            )
            # psB: qk-proj groups + score chunks ([128,1024] f32 = 2 banks x 3)
            psB = ctx.enter_context(
                tc.tile_pool(name="psB", bufs=3, space="PSUM")
            )
            rp = ctx.enter_context(tc.tile_pool(name="rope", bufs=6))
            ep = ctx.enter_context(tc.tile_pool(name="ep", bufs=4))
            dp = ctx.enter_context(tc.tile_pool(name="dp", bufs=2))
            dtp = ctx.enter_context(tc.tile_pool(name="dtp", bufs=3))
            st = ctx.enter_context(tc.tile_pool(name="st", bufs=32))
            op = ctx.enter_context(tc.tile_pool(name="op", bufs=2))

            wqk_s = pers.tile([128, NCC * 4 * HS], BF16)  # chunk c at 512c
            wv_s = pers.tile([128, NCC * E2], BF16)       # chunk c at 256c
            cos_s = pers.tile([128, T], BF16)
            sin_s = pers.tile([128, T], BF16)
            msk_s = pers.tile([128, 2 * HS], BF16)        # [I | -30*triu]
            lam_s = pers.tile([128, 2], F32)              # [c0, c1/c0]
            xt_s = pers.tile([128, NCC, T], BF16, name="xt")
            # q/k tensors, tau: 0=q0 1=k0 2=q1 3=k1 (post-RoPE, [d', t])
            q_t = [
                pers.tile([128, T], BF16, name=f"q{t}", tag=f"q{t}")
                for t in range(4)
            ]
            v_t = [
                pers.tile([128, E2], BF16, name=f"v{j}", tag=f"v{j}")
                for j in range(NTILE)
            ]

            # ---- input DMAs, split across both hwdge queues ----
            # per-queue transfers serialize (~180GB/s); x (4MB) gates the
            # projection, so it is split ~evenly and leads on both queues
            # right after wqk (needed by the very first matmul).
            # wqk in two halves: [q0|k0] cols first so the first two
            # projection passes can start before [q1|k1] lands.
            wqk_v = wqk_s[:].rearrange("p (c w) -> p c w", c=NCC)
            nc.sync.dma_start(
                wqk_v[:, :, 0 : 2 * HS],
                wqk[:, 0 : 2 * HS].rearrange("(c p) w -> p c w", c=NCC),
            )
            for c in range(3):
                nc.sync.dma_start(xt_s[:, c, :], xT[128 * c : 128 * (c + 1), :])
            for c in range(3, NCC):
                nc.scalar.dma_start(xt_s[:, c, :], xT[128 * c : 128 * (c + 1), :])
            nc.sync.dma_start(
                wqk_v[:, :, 2 * HS : 4 * HS],
                wqk[:, 2 * HS : 4 * HS].rearrange("(c p) w -> p c w", c=NCC),
            )
            nc.sync.dma_start(msk_s[:], cmask[:])
            nc.sync.dma_start(lam_s[:], lamc[:])
            nc.sync.dma_start(cos_s[:], cosb[:])
            nc.scalar.dma_start(sin_s[:], sinb[:])
            nc.scalar.dma_start(
                wv_s[:].rearrange("p (c w) -> p c w", c=NCC),
                wv[:].rearrange("(c p) w -> p c w", c=NCC),
            )

            i_ap = msk_s[:, 0:128]
            u_ap = msk_s[:, 128:256]

            # ---- q/k projection + RoPE, per (tau, 1024-col group) ----
            def proj_qk(tau, g):
                pj = psB.tile([128, 1024], F32, tag="sp")
                for c in range(NCC):
                    w_ap = wqk_s[:, 512 * c + 128 * tau : 512 * c + 128 * (tau + 1)]
                    nc.tensor.matmul(
                        pj[:, 0:512],
                        w_ap,
                        xt_s[:, c, 1024 * g : 1024 * g + 512],
                        start=(c == 0),
                        stop=(c == NCC - 1),
                        skip_group_check=True,
                    )
                    nc.tensor.matmul(
                        pj[:, 512:1024],
                        w_ap,
                        xt_s[:, c, 1024 * g + 512 : 1024 * (g + 1)],
                        start=(c == 0),
                        stop=(c == NCC - 1),
                        skip_group_check=True,
                    )
                sl = slice(1024 * g, 1024 * (g + 1))
                t1 = rp.tile([128, 1024], BF16, tag="t1")
                nc.vector.tensor_mul(t1[:], pj[:], cos_s[:, sl])
                u = rp.tile([128, 1024], BF16, tag="u")
                nc.vector.tensor_mul(u[:], pj[:], sin_s[:, sl])
                usw = rp.tile([128, 1024], BF16, tag="usw")
                nc.scalar.dma_start(usw[0:64, :], u[64:128, :])
                nc.scalar.dma_start(usw[64:128, :], u[0:64, :])
                # alternate the add between DVE and Pool to keep DVE free
                eng = nc.gpsimd if (2 * tau + g) % 2 == 0 else nc.vector
                eng.tensor_add(q_t[tau][:, sl], t1[:], usw[:])

            for tau in range(4):
                for g in range(2):
                    proj_qk(tau, g)

            # ---- v projection for s-block j ----
            def proj_v(j):
                vp = psA.tile([128, 512], F32, tag="av")
                for c in range(NCC):
                    nc.tensor.matmul(
                        vp[:, :E2],
                        xt_s[:, c, 128 * j : 128 * (j + 1)],
                        wv_s[:, E2 * c : E2 * (c + 1)],
                        start=(c == 0),
                        stop=(c == NCC - 1),
                        skip_group_check=True,
                    )
                nc.vector.tensor_copy(v_t[j][:], vp[:, :E2])

            # ---- scores + exp + D + D^T for row-tile i ----
            dts = {}

            def scores_part(i):
                W = 128 * (i + 1)
                nch = (W + 1023) // 1024
                es, ls = [], []
                for n in range(2):
                    en = ep.tile([128, T], BF16, tag=f"E{n}")
                    lp = st.tile([128, 2], F32, tag=f"lp{n}")
                    for ch in range(nch):
                        off = 1024 * ch
                        wch = min(1024, W - off)
                        sp = psB.tile([128, 1024], F32, tag="sp")
                        for sub in range(0, wch, 512):
                            wsub = min(512, wch - sub)
                            diag = off + sub + wsub == W
                            nc.tensor.matmul(
                                sp[:, sub : sub + wsub],
                                q_t[2 * n][:, 128 * i : 128 * (i + 1)],
                                q_t[2 * n + 1][:, off + sub : off + sub + wsub],
                                start=True,
                                stop=not diag,
                                skip_group_check=True,
                            )
                            if diag:
                                nc.tensor.matmul(
                                    sp[:, sub + wsub - 128 : sub + wsub],
                                    i_ap,
                                    u_ap,
                                    start=False,
                                    stop=True,
                                    skip_group_check=True,
                                )
                        nc.scalar.activation(
                            en[:, off : off + wch],
                            sp[:, :wch],
                            AF.Exp,
                            accum_out=lp[:, ch : ch + 1],
                        )
                    if nch == 1:
                        ln_ap = lp[:, 0:1]
                    else:
                        ln = st.tile([128, 1], F32, tag=f"l{n}")
                        nc.vector.tensor_add(ln[:], lp[:, 0:1], lp[:, 1:2])
                        ln_ap = ln[:]
                    es.append(en)
                    ls.append(ln_ap)

                r1 = st.tile([128, 1], F32, tag="r1")
                nc.vector.reciprocal(r1[:], ls[1])
                beta = st.tile([128, 1], F32, tag="beta")
                nc.vector.tensor_scalar(
                    beta[:], ls[0], r1[:], lam_s[:, 1:2], ALU.mult, ALU.mult
                )
                r0 = st.tile([128, 1], F32, tag="r0")
                nc.vector.reciprocal(r0[:], ls[0])
                alpha = st.tile([128, 1], F32, tag="alpha")
                nc.vector.tensor_mul(alpha[:], r0[:], lam_s[:, 0:1])

                d = dp.tile([128, T], BF16, tag="d")
                dt = dtp.tile([128, NTILE, 128], BF16, tag="dt")
                for ch in range(nch):
                    off = 1024 * ch
                    wch = min(1024, W - off)
                    nc.vector.scalar_tensor_tensor(
                        d[:, off : off + wch],
                        es[1][:, off : off + wch],
                        beta[:],
                        es[0][:, off : off + wch],
                        ALU.mult,
                        ALU.add,
                    )
                    nc.sync.dma_start_transpose(
                        dt[:, 8 * ch : 8 * ch + wch // 128, :],
                        d[:, off : off + wch],
                    )
                dts[i] = (dt, alpha)

            # ---- AV + out for row-tile i ----
            def av_part(i):
                dt, alpha = dts.pop(i)
                av = psA.tile([128, 512], F32, tag="av")
                for j in range(i + 1):
                    nc.tensor.matmul(
                        av[:, :E2],
                        dt[:, j, :],
                        v_t[j][:],
                        start=(j == 0),
                        stop=(j == i),
                        skip_group_check=True,
                    )
                ot = op.tile([128, E2], F32, tag="ot")
                nc.vector.tensor_scalar(
                    ot[:], av[:, :E2], alpha[:], None, ALU.mult
                )
                nc.sync.dma_start(outp[128 * i : 128 * (i + 1), :], ot[:])

            # AV runs two row-tiles behind scores so the PE never waits on
            # the exp -> beta -> D -> D^T chain.  v-projections are packed
            # into the early (small) iterations as PE filler, keeping the
            # steady-state loop to scores+AV on PE and stats+combine on DVE.
            for i in range(NTILE):
                proj_v(i)
                scores_part(i)
                if i > 1:
                    av_part(i - 2)
            av_part(NTILE - 2)
            av_part(NTILE - 1)

    nc.compile()
    return nc


_CACHE = {}


def _get_nc():
    if "nc" not in _CACHE:
        _CACHE["nc"] = build_nc()
    return _CACHE["nc"]


def _prep_host(x, Wq, Wk, Wv, lambda_q, lambda_k, layer_idx):
    bf = ml_dtypes.bfloat16
    perm = np.concatenate([np.arange(0, HS, 2), np.arange(1, HS, 2)])
    scale = 1.0 / np.sqrt(HS)
    Wqp = np.asarray(Wq, np.float32)[:, :, perm] * scale
    Wkp = np.asarray(Wk, np.float32)[:, :, perm]
    # tau order: q0, k0, q1, k1
    wqk = np.concatenate([Wqp[0], Wkp[0], Wqp[1], Wkp[1]], axis=1).astype(bf)
    wv = np.asarray(Wv, np.float32).astype(bf)

    f = 1.0 / THETA ** (np.arange(0, HS, 2, dtype=np.float64) / HS)
    ang = np.outer(f, np.arange(T, dtype=np.float64))  # [64, T]
    cosb = np.concatenate([np.cos(ang), np.cos(ang)], 0).astype(bf)
    # swap-after-mul RoPE: u = qb * sinb; usw = partition-swap(u);
    # out = qb*cos + usw.  Want usw[0:64] = -sin*xi, usw[64:128] = +sin*xr
    # => sinb rows = [+sin ; -sin]
    sinb = np.concatenate([np.sin(ang), -np.sin(ang)], 0).astype(bf)

    eye = np.eye(HS, dtype=np.float32)
    u30 = NEG * np.triu(np.ones((HS, HS), np.float32), k=1)
    cmask = np.concatenate([eye, u30], axis=1).astype(bf)

    li = float(np.asarray(layer_idx))
    lam_init = 0.8 - 0.6 * np.exp(-0.3 * (li - 1.0))
    e = np.mean(
        np.exp(np.asarray(lambda_q, np.float32) * np.asarray(lambda_k, np.float32)),
        axis=-1,
    )
    lam = e - np.concatenate([[0.0], e[:-1]]) + lam_init
    c0, c1 = float(lam[0]), float(-lam[1])
    lamc = np.tile(np.array([[c0, c1 / c0]], np.float32), (HS, 1))

    xT = np.ascontiguousarray(np.asarray(x, np.float32).transpose(0, 2, 1)).astype(bf)
    return xT, wqk, wv, cosb, sinb, cmask, lamc


def _make_in_maps(np_inputs):
    xT, wqk, wv, cosb, sinb, cmask, lamc = _prep_host(**np_inputs)
    return [
        {
            "xT": xT[b],
            "wqk": wqk,
            "wv": wv,
            "cosb": cosb,
            "sinb": sinb,
            "cmask": cmask,
            "lamc": lamc,
        }
        for b in range(B)
    ]


def _collect(res):
    return np.stack([res.results[b]["out"] for b in range(B)]).astype(np.float32)


def kernel(x, Wq, Wk, Wv, lambda_q, lambda_k, layer_idx):
    from concourse.bass_utils import run_bass_kernel_spmd

    in_maps = _make_in_maps(
        dict(x=x, Wq=Wq, Wk=Wk, Wv=Wv, lambda_q=lambda_q,
             lambda_k=lambda_k, layer_idx=layer_idx)
    )
    res = run_bass_kernel_spmd(_get_nc(), in_maps, core_ids=list(range(B)))
    return _collect(res)


# revision 21
# speedup vs baseline: 1.0821x; 1.0821x over previous
"""AlternatingDiffHead Trainium2 kernel.

Data-parallel over batch: B=8 batch elements -> 8 NeuronCores, one batch
element per core, no collectives.

Per-core math (T=2048, C=1024, HS=128, 2 terms):
  v  = x @ Wv                                  [T, 256]
  qn = rope(x @ Wqn * 1/sqrt(HS)),  kn = rope(x @ Wkn)     [T, 128]
  Sn = qn @ kn^T  (causal)                      [T, T]
  En = exp(Sn)    (no max-sub; S is O(1))       rowsum -> ln
  D  = E0 + beta E1,  beta[t] = (c1 l0[t]) / (c0 l1[t])
  out[t] = (c0 / l0[t]) * (D @ v)[t]
where c0 = lam0, c1 = -lam1 (host-computed scalars).

v3 design notes:
 - i-major single pass: full q/k projection+RoPE first (PE stays dense),
   then one loop over the 16 row-tiles doing scores(term0+term1) -> exp
   -> beta -> D -> D^T -> AV -> out, with v-projection interleaved.
 - AV is software-pipelined one row-tile behind scores so the PE never
   waits on the exp -> beta -> D -> D^T chain.
 - D^T produced by ONE dma_start_transpose (XBAR) per row-tile into a
   [128, 16, 128] tile, replacing 136 PE transposes + PSUM->SBUF copies.
 - D combine is ONE scalar_tensor_tensor per row-tile:
   D = (E1 * beta) + E0.
 - RoPE rotate-half swap moved AFTER the sin-multiply (swap the product,
   not the input), so the PSUM->bf16 cast is fused into the cos/sin muls
   (vector reads PSUM directly); sin sign pattern pre-swapped on host.
 - DMAs split across the two hwdge queues (sync + scalar): x is loaded
   4 chunks per queue in parallel; RoPE swaps go to the scalar queue
   (idle during projection), D-transposes + output to sync.
 - Engine balance in the loop: PE scores/v/AV; ACT exp only; DVE
   D-combine, softmax stats, v copies, out scale.
"""

import numpy as np
import ml_dtypes
from contextlib import ExitStack

import concourse.bass as bass
import concourse.tile as tile
from concourse import bacc, mybir

B, T, C, HS, NT = 8, 2048, 1024, 128, 2
E2 = 2 * HS  # v/out feature dim (256)
THETA = 10000.0
NEG = -30.0
BF16, F32 = mybir.dt.bfloat16, mybir.dt.float32
AF = mybir.ActivationFunctionType
ALU = mybir.AluOpType
NCC = C // 128         # 8 contraction chunks
NTILE = T // 128       # 16 row tiles


def build_nc():
    nc = bacc.Bacc("TRN2", target_bir_lowering=False, debug=False, num_devices=8)

    xT = nc.declare_dram_parameter("xT", [C, T], BF16, isOutput=False)
    wqk = nc.declare_dram_parameter("wqk", [C, 4 * HS], BF16, isOutput=False)
    wv = nc.declare_dram_parameter("wv", [C, E2], BF16, isOutput=False)
    cosb = nc.declare_dram_parameter("cosb", [HS, T], BF16, isOutput=False)
    sinb = nc.declare_dram_parameter("sinb", [HS, T], BF16, isOutput=False)
    cmask = nc.declare_dram_parameter("cmask", [HS, 2 * HS], BF16, isOutput=False)
    lamc = nc.declare_dram_parameter("lamc", [HS, 2], F32, isOutput=False)
    outp = nc.declare_dram_parameter("out", [T, E2], F32, isOutput=True)

    with tile.TileContext(nc) as tc:
        with ExitStack() as ctx:
            pers = ctx.enter_context(tc.tile_pool(name="pers", bufs=1))
            # psA: v-proj + AV accum ([128,512] f32 = 1 bank x 2)
            psA = ctx.enter_context(
                tc.tile_pool(name="psA", bufs=2, space="PSUM")
            )
            # psB: qk-proj groups + score chunks ([128,1024] f32 = 2 banks x 3)
            psB = ctx.enter_context(
                tc.tile_pool(name="psB", bufs=3, space="PSUM")
            )
            rp = ctx.enter_context(tc.tile_pool(name="rope", bufs=6))
            ep = ctx.enter_context(tc.tile_pool(name="ep", bufs=4))
            dp = ctx.enter_context(tc.tile_pool(name="dp", bufs=2))
            dtp = ctx.enter_context(tc.tile_pool(name="dtp", bufs=3))
            st = ctx.enter_context(tc.tile_pool(name="st", bufs=32))
            op = ctx.enter_context(tc.tile_pool(name="op", bufs=2))

            wqk_s = pers.tile([128, NCC * 4 * HS], BF16)  # chunk c at 512c
            wv_s = pers.tile([128, NCC * E2], BF16)       # chunk c at 256c
            cos_s = pers.tile([128, T], BF16)
            sin_s = pers.tile([128, T], BF16)
            msk_s = pers.tile([128, 2 * HS], BF16)        # [I | -30*triu]
            lam_s = pers.tile([128, 2], F32)              # [c0, c1/c0]
            xt_s = pers.tile([128, NCC, T], BF16, name="xt")
            # q/k tensors, tau: 0=q0 1=k0 2=q1 3=k1 (post-RoPE, [d', t])
            q_t = [
                pers.tile([128, T], BF16, name=f"q{t}", tag=f"q{t}")
                for t in range(4)
            ]
            v_t = [
                pers.tile([128, E2], BF16, name=f"v{j}", tag=f"v{j}")
                for j in range(NTILE)
            ]

            # ---- input DMAs, split across both hwdge queues ----
            # per-queue transfers serialize (~180GB/s); x (4MB) gates the
            # projection, so it is split ~evenly and leads on both queues
            # right after wqk (needed by the very first matmul).
            nc.sync.dma_start(
                wqk_s[:].rearrange("p (c w) -> p c w", c=NCC),
                wqk[:].rearrange("(c p) w -> p c w", c=NCC),
            )
            for c in range(3):
                nc.sync.dma_start(xt_s[:, c, :], xT[128 * c : 128 * (c + 1), :])
            for c in range(3, NCC):
                nc.scalar.dma_start(xt_s[:, c, :], xT[128 * c : 128 * (c + 1), :])
            nc.sync.dma_start(msk_s[:], cmask[:])
            nc.sync.dma_start(lam_s[:], lamc[:])
            nc.sync.dma_start(cos_s[:], cosb[:])
            nc.scalar.dma_start(sin_s[:], sinb[:])
            nc.scalar.dma_start(
                wv_s[:].rearrange("p (c w) -> p c w", c=NCC),
                wv[:].rearrange("(c p) w -> p c w", c=NCC),
            )

            i_ap = msk_s[:, 0:128]
            u_ap = msk_s[:, 128:256]

            # ---- q/k projection + RoPE, per (tau, 1024-col group) ----
            def proj_qk(tau, g):
                pj = psB.tile([128, 1024], F32, tag="sp")
                for c in range(NCC):
                    w_ap = wqk_s[:, 512 * c + 128 * tau : 512 * c + 128 * (tau + 1)]
                    nc.tensor.matmul(
                        pj[:, 0:512],
                        w_ap,
                        xt_s[:, c, 1024 * g : 1024 * g + 512],
                        start=(c == 0),
                        stop=(c == NCC - 1),
                        skip_group_check=True,
                    )
                    nc.tensor.matmul(
                        pj[:, 512:1024],
                        w_ap,
                        xt_s[:, c, 1024 * g + 512 : 1024 * (g + 1)],
                        start=(c == 0),
                        stop=(c == NCC - 1),
                        skip_group_check=True,
                    )
                sl = slice(1024 * g, 1024 * (g + 1))
                t1 = rp.tile([128, 1024], BF16, tag="t1")
                nc.vector.tensor_mul(t1[:], pj[:], cos_s[:, sl])
                u = rp.tile([128, 1024], BF16, tag="u")
                nc.vector.tensor_mul(u[:], pj[:], sin_s[:, sl])
                usw = rp.tile([128, 1024], BF16, tag="usw")
                nc.scalar.dma_start(usw[0:64, :], u[64:128, :])
                nc.scalar.dma_start(usw[64:128, :], u[0:64, :])
                # alternate the add between DVE and Pool to keep DVE free
                eng = nc.gpsimd if (2 * tau + g) % 2 == 0 else nc.vector
                eng.tensor_add(q_t[tau][:, sl], t1[:], usw[:])

            for tau in range(4):
                for g in range(2):
                    proj_qk(tau, g)

            # ---- v projection for s-block j ----
            def proj_v(j):
                vp = psA.tile([128, 512], F32, tag="av")
                for c in range(NCC):
                    nc.tensor.matmul(
                        vp[:, :E2],
                        xt_s[:, c, 128 * j : 128 * (j + 1)],
                        wv_s[:, E2 * c : E2 * (c + 1)],
                        start=(c == 0),
                        stop=(c == NCC - 1),
                        skip_group_check=True,
                    )
                nc.vector.tensor_copy(v_t[j][:], vp[:, :E2])

            # ---- scores + exp + D + D^T for row-tile i ----
            dts = {}

            def scores_part(i):
                W = 128 * (i + 1)
                nch = (W + 1023) // 1024
                es, ls = [], []
                for n in range(2):
                    en = ep.tile([128, T], BF16, tag=f"E{n}")
                    lp = st.tile([128, 2], F32, tag=f"lp{n}")
                    for ch in range(nch):
                        off = 1024 * ch
                        wch = min(1024, W - off)
                        sp = psB.tile([128, 1024], F32, tag="sp")
                        for sub in range(0, wch, 512):
                            wsub = min(512, wch - sub)
                            diag = off + sub + wsub == W
                            nc.tensor.matmul(
                                sp[:, sub : sub + wsub],
                                q_t[2 * n][:, 128 * i : 128 * (i + 1)],
                                q_t[2 * n + 1][:, off + sub : off + sub + wsub],
                                start=True,
                                stop=not diag,
                                skip_group_check=True,
                            )
                            if diag:
                                nc.tensor.matmul(
                                    sp[:, sub + wsub - 128 : sub + wsub],
                                    i_ap,
                                    u_ap,
                                    start=False,
                                    stop=True,
                                    skip_group_check=True,
                                )
                        nc.scalar.activation(
                            en[:, off : off + wch],
                            sp[:, :wch],
                            AF.Exp,
                            accum_out=lp[:, ch : ch + 1],
                        )
                    if nch == 1:
                        ln_ap = lp[:, 0:1]
                    else:
                        ln = st.tile([128, 1], F32, tag=f"l{n}")
                        nc.vector.tensor_add(ln[:], lp[:, 0:1], lp[:, 1:2])
                        ln_ap = ln[:]
                    es.append(en)
                    ls.append(ln_ap)

                r1 = st.tile([128, 1], F32, tag="r1")
                nc.vector.reciprocal(r1[:], ls[1])
                beta = st.tile([128, 1], F32, tag="beta")
                nc.vector.tensor_scalar(
                    beta[:], ls[0], r1[:], lam_s[:, 1:2], ALU.mult, ALU.mult
                )
                r0 = st.tile([128, 1], F32, tag="r0")
                nc.vector.reciprocal(r0[:], ls[0])
                alpha = st.tile([128, 1], F32, tag="alpha")
                nc.vector.tensor_mul(alpha[:], r0[:], lam_s[:, 0:1])

                d = dp.tile([128, T], BF16, tag="d")
                dt = dtp.tile([128, NTILE, 128], BF16, tag="dt")
                for ch in range(nch):
                    off = 1024 * ch
                    wch = min(1024, W - off)
                    nc.vector.scalar_tensor_tensor(
                        d[:, off : off + wch],
                        es[1][:, off : off + wch],
                        beta[:],
                        es[0][:, off : off + wch],
                        ALU.mult,
                        ALU.add,
                    )
                    nc.sync.dma_start_transpose(
                        dt[:, 8 * ch : 8 * ch + wch // 128, :],
                        d[:, off : off + wch],
                    )
                dts[i] = (dt, alpha)

            # ---- AV + out for row-tile i ----
            def av_part(i):
                dt, alpha = dts.pop(i)
                av = psA.tile([128, 512], F32, tag="av")
                for j in range(i + 1):
                    nc.tensor.matmul(
                        av[:, :E2],
                        dt[:, j, :],
                        v_t[j][:],
                        start=(j == 0),
                        stop=(j == i),
                        skip_group_check=True,
                    )
                ot = op.tile([128, E2], F32, tag="ot")
                nc.vector.tensor_scalar(
                    ot[:], av[:, :E2], alpha[:], None, ALU.mult
                )
                nc.sync.dma_start(outp[128 * i : 128 * (i + 1), :], ot[:])

            # AV runs two row-tiles behind scores so the PE never waits on
            # the exp -> beta -> D -> D^T chain.  v-projections are packed
            # into the early (small) iterations as PE filler, keeping the
            # steady-state loop to scores+AV on PE and stats+combine on DVE.
            for i in range(NTILE):
                proj_v(i)
                scores_part(i)
                if i > 1:
                    av_part(i - 2)
            av_part(NTILE - 2)
            av_part(NTILE - 1)

    nc.compile()
    return nc


_CACHE = {}


def _get_nc():
    if "nc" not in _CACHE:
        _CACHE["nc"] = build_nc()
    return _CACHE["nc"]


def _prep_host(x, Wq, Wk, Wv, lambda_q, lambda_k, layer_idx):
    bf = ml_dtypes.bfloat16
    perm = np.concatenate([np.arange(0, HS, 2), np.arange(1, HS, 2)])
    scale = 1.0 / np.sqrt(HS)
    Wqp = np.asarray(Wq, np.float32)[:, :, perm] * scale
    Wkp = np.asarray(Wk, np.float32)[:, :, perm]
    # tau order: q0, k0, q1, k1
    wqk = np.concatenate([Wqp[0], Wkp[0], Wqp[1], Wkp[1]], axis=1).astype(bf)
    wv = np.asarray(Wv, np.float32).astype(bf)

    f = 1.0 / THETA ** (np.arange(0, HS, 2, dtype=np.float64) / HS)
    ang = np.outer(f, np.arange(T, dtype=np.float64))  # [64, T]
    cosb = np.concatenate([np.cos(ang), np.cos(ang)], 0).astype(bf)
    # swap-after-mul RoPE: u = qb * sinb; usw = partition-swap(u);
    # out = qb*cos + usw.  Want usw[0:64] = -sin*xi, usw[64:128] = +sin*xr
    # => sinb rows = [+sin ; -sin]
    sinb = np.concatenate([np.sin(ang), -np.sin(ang)], 0).astype(bf)

    eye = np.eye(HS, dtype=np.float32)
    u30 = NEG * np.triu(np.ones((HS, HS), np.float32), k=1)
    cmask = np.concatenate([eye, u30], axis=1).astype(bf)

    li = float(np.asarray(layer_idx))
    lam_init = 0.8 - 0.6 * np.exp(-0.3 * (li - 1.0))
    e = np.mean(
        np.exp(np.asarray(lambda_q, np.float32) * np.asarray(lambda_k, np.float32)),
        axis=-1,
    )
    lam = e - np.concatenate([[0.0], e[:-1]]) + lam_init
    c0, c1 = float(lam[0]), float(-lam[1])
    lamc = np.tile(np.array([[c0, c1 / c0]], np.float32), (HS, 1))

    xT = np.ascontiguousarray(np.asarray(x, np.float32).transpose(0, 2, 1)).astype(bf)
    return xT, wqk, wv, cosb, sinb, cmask, lamc


def _make_in_maps(np_inputs):
    xT, wqk, wv, cosb, sinb, cmask, lamc = _prep_host(**np_inputs)
    return [
        {
            "xT": xT[b],
            "wqk": wqk,
            "wv": wv,
            "cosb": cosb,
            "sinb": sinb,
            "cmask": cmask,
            "lamc": lamc,
        }
        for b in range(B)
    ]


def _collect(res):
    return np.stack([res.results[b]["out"] for b in range(B)]).astype(np.float32)


def kernel(x, Wq, Wk, Wv, lambda_q, lambda_k, layer_idx):
    from concourse.bass_utils import run_bass_kernel_spmd

    in_maps = _make_in_maps(
        dict(x=x, Wq=Wq, Wk=Wk, Wv=Wv, lambda_q=lambda_q,
             lambda_k=lambda_k, layer_idx=layer_idx)
    )
    res = run_bass_kernel_spmd(_get_nc(), in_maps, core_ids=list(range(B)))
    return _collect(res)
